# revision 2
# baseline (speedup 1.0000x reference)
"""Trainium2 Bass kernel for nn_AffinityImageEvent.

Math: the reference L2-normalizes image/event over C, then for each of the
9 offsets (i,j) of a 3x3 window computes sum_c img_shift*evt_shift -- both
tensors shifted by the SAME offset.  That means every output channel k is
just a shifted copy of the zero-padded per-pixel cosine map

    D[b,h,w] = (img . evt) / (||img|| ||evt||)        (over C=128)
    out[b, k=(i,j), h, w] = relu(Dpad[b, h+i, w+j])

So the kernel computes three C-reductions per pixel (img.evt, img^2, evt^2),
a tiny pointwise epilogue, and 9 shifted DMA stores.

Sharding: B(4) x H-halves(2) -> 8 cores, each core gets [C=128, 98, 256]
(96 rows + 1 halo row each side, zero-padded at image boundaries).

The f32 version of this kernel sat exactly on the f32 input-DMA roofline
(25.7 MB/core @ ~360 GB/s = 71.4 us).  Inputs are therefore staged to HBM
as f16 (host-side astype; quantization adds ~3.6e-4 L2 error vs the 2e-2
budget), halving mandatory traffic: 13.7 MB/core => ~37 us DMA bound.
f16 also makes every PE matmul full-rate (1 col/cycle) and enables the
DVE 2x 16-bit mode, keeping all compute under the DMA shadow:

Per-core pipeline:
  - stream input in 7-row chunks [128, 1792] f16 via HWDGE
  - elementwise: prod=img*evt and img^2 on DVE (2x 16-bit), evt^2 on ACT,
    squares written interleaved per row into one [128, 2*1792] f16 tile
  - PE: per row r, sliding one-hot ones-column lhsT (A[:, 98-r:196-r], col
    r hot) reduces [128C, 256W] into PSUM partition r; s1 from prod
    ([128, 256] f16, full rate), s2|s3 share one [128, 512] f16 matmul.
  - epilogue: D = relu(s1) * rsqrt(s2*s3 + eps)  (sqrt on ACT, reciprocal
    on DVE), written into a zero-framed [98, 258] tile
  - 9 shifted DMAs [96, 256] f32 -> out[k]
"""

import sys

sys.path.insert(0, "/opt/trn_rl_repo")

import numpy as np

try:
    import jax

    jax.config.update("jax_compilation_cache_dir", "/tmp/affinity_jaxcache")
    jax.config.update("jax_persistent_cache_min_compile_time_secs", 1.0)
    jax.config.update("jax_persistent_cache_min_entry_size_bytes", 0)
except Exception:
    pass

import concourse.bass as bass
import concourse.bacc as bacc
import concourse.tile as tile
from concourse import mybir
from concourse.bass_utils import run_bass_kernel_spmd

B, C, H, W = 4, 128, 192, 256
KWIN = 9
N_CORES = 8
HALF = H // 2              # 96 output rows per core
ROWS = HALF + 2            # 98 D rows incl. halo
# chunk row-count schedule: small first chunk gets PE started early; a
# gradual taper at the end collapses the products->matmul pipeline phase
# lag (~ one chunk of work) so the epilogue starts right after the last DMA.
CHUNK_SCHED = [2] + [7] * 10 + [6, 5, 4, 3, 2, 2, 2, 1, 1]
assert sum(CHUNK_SCHED) == ROWS
# PSUM row-group boundary.  Splitting lets group-0's epilogue and output
# DMAs overlap with group-1's matmul stream.
GSPLIT = 86
MAXCW = max(CHUNK_SCHED) * W
IN_BUFS = 3
PROD_BUFS = 3

F32 = mybir.dt.float32
F16 = mybir.dt.float16
AF = mybir.ActivationFunctionType

EPS = 1e-30                # keeps zero halo rows finite (0 * big = 0)


def build_program(repeat: int = 1) -> bass.Bass:
    nc = bacc.Bacc("TRN2", target_bir_lowering=False, debug=False)
    img_d = nc.dram_tensor("image", [C, ROWS * W], F16, kind="ExternalInput").ap()
    evt_d = nc.dram_tensor("event", [C, ROWS * W], F16, kind="ExternalInput").ap()
    out_d = nc.dram_tensor("out", [KWIN, HALF, W], F32, kind="ExternalOutput").ap()

    # sliding one-hot: A[:, 98-r : 196-r] has its ones-column at position r
    # -> matmul writes row-r sums to PSUM partition r.
    A16 = nc.alloc_sbuf_tensor("onehot", [C, 2 * ROWS], F16).ap()

    with tile.TileContext(nc) as tc:
        with (
            tc.tile_pool(name="inp", bufs=IN_BUFS) as ipool,
            tc.tile_pool(name="prod", bufs=PROD_BUFS) as ppool,
            tc.tile_pool(name="acc", bufs=2, space="PSUM") as psum,
            tc.tile_pool(name="epi", bufs=2) as epool,
        ):
            nc.gpsimd.memset(A16[:, 0:ROWS], 0.0)
            nc.gpsimd.memset(A16[:, ROWS : ROWS + 1], 1.0)
            nc.gpsimd.memset(A16[:, ROWS + 1 : 2 * ROWS], 0.0)
            eps_t = nc.alloc_sbuf_tensor("eps", [C, 1], F32).ap()
            nc.gpsimd.memset(eps_t, EPS)
            # dummy Sqrt up front nudges the act-table pass to load a
            # sqrt-capable set (sqrt_and_* also contain Square/Relu/Copy),
            # avoiding a mid-epilogue table switch
            warm_t = nc.alloc_sbuf_tensor("actwarm", [C, 1], F32).ap()
            nc.scalar.activation(warm_t, eps_t, AF.Sqrt)

            for _ in range(repeat):
                # two independent PSUM row-groups: group 0's epilogue and
                # output DMAs overlap with group 1's matmul stream
                gsz = [GSPLIT, ROWS - GSPLIT]
                ngroups = 2 if gsz[1] > 0 else 1
                s1g = [
                    psum.tile([C, W], F32, tag=f"s1g{g}", name=f"s1g{g}")
                    for g in range(ngroups)
                ]
                s23g = [
                    psum.tile([C, 2 * W], F32, tag=f"s23g{g}", name=f"s23g{g}")
                    for g in range(ngroups)
                ]

                r0 = 0
                for k, crows in enumerate(CHUNK_SCHED):
                    cw = crows * W
                    cs0 = r0 * W
                    img_t = ipool.tile([C, MAXCW], F16, tag="img")
                    nc.sync.dma_start(
                        out=img_t[:, 0:cw], in_=img_d[:, cs0 : cs0 + cw]
                    )
                    evt_t = ipool.tile([C, MAXCW], F16, tag="evt")
                    # NOTE: keep both input DMAs on nc.sync — issuing evt on
                    # the ACT ring queues DMA issues behind blocked Square
                    # dispatches and regresses throughput
                    nc.sync.dma_start(
                        out=evt_t[:, 0:cw], in_=evt_d[:, cs0 : cs0 + cw]
                    )

                    prod = ppool.tile([C, MAXCW], F16, tag="prod")
                    nc.vector.tensor_mul(
                        prod[:, 0:cw], img_t[:, 0:cw], evt_t[:, 0:cw]
                    )

                    # squares interleaved per row: [sqi_row | sqe_row] so one
                    # f16 matmul covers both norms
                    sq = ppool.tile([C, 2 * MAXCW], F16, tag="sq")
                    sqv = sq[:, 0 : 2 * cw].rearrange(
                        "c (q x) -> c q x", x=2 * W
                    )
                    img3 = img_t[:, 0:cw].rearrange("c (q w) -> c q w", w=W)
                    evt3 = evt_t[:, 0:cw].rearrange("c (q w) -> c q w", w=W)
                    nc.vector.tensor_mul(sqv[:, :, 0:W], img3, img3)
                    nc.scalar.activation(sqv[:, :, W : 2 * W], evt3, AF.Square)

                    for q in range(crows):
                        r = r0 + q
                        g = 0 if r < GSPLIT else 1
                        pos = r - g * GSPLIT
                        m = gsz[g]
                        st = pos == 0
                        sp = pos == m - 1
                        lt = slice(ROWS - pos, 2 * ROWS - pos - (ROWS - m))
                        qs = slice(q * W, (q + 1) * W)
                        nc.tensor.matmul(
                            s1g[g][0:m, :], A16[:, lt], prod[:, qs],
                            start=st, stop=sp,
                        )
                        nc.tensor.matmul(
                            s23g[g][0:m, :],
                            A16[:, lt],
                            sq[:, q * 2 * W : (q + 1) * 2 * W],
                            start=st,
                            stop=sp,
                        )
                    r0 += crows

                # per-group epilogue: D = relu(s1) * rsqrt(s2*s3 + eps).
                # group 0 runs as soon as its last matmul retires,
                # overlapping with group 1's matmul stream.
                out4 = out_d.rearrange("(i j) h w -> i j h w", i=3)
                for g in range(ngroups):
                    m = gsz[g]
                    rp = slice(0, m)
                    s1_t, s23_t = s1g[g], s23g[g]
                    s2sb = epool.tile([C, W], F32, tag=f"s2sb{g}")
                    nc.scalar.activation(s2sb[rp, :], s23_t[rp, 0:W], AF.Copy)
                    t23 = epool.tile([C, W], F32, tag=f"t23{g}")
                    nc.vector.tensor_mul(
                        t23[rp, :], s2sb[rp, :], s23_t[rp, W : 2 * W]
                    )
                    sqr = epool.tile([C, W], F32, tag=f"sqr{g}")
                    nc.scalar.activation(
                        sqr[rp, :], t23[rp, :], AF.Sqrt, bias=eps_t[rp]
                    )
                    y = epool.tile([C, W], F32, tag=f"y{g}")
                    nc.vector.reciprocal(y[rp, :], sqr[rp, :])
                    s1r = epool.tile([C, W], F32, tag=f"s1r{g}")
                    nc.scalar.activation(s1r[rp, :], s1_t[rp, :], AF.Relu)

                    dpad = epool.tile([C, W + 2], F32, tag=f"dpad{g}")
                    nc.vector.memset(dpad[rp, 0:1], 0.0)
                    nc.vector.memset(dpad[rp, W + 1 : W + 2], 0.0)
                    nc.vector.tensor_mul(
                        dpad[rp, 1 : W + 1], s1r[rp, :], y[rp, :]
                    )

                    # shifted outputs: one DMA per window-row i covers the 3
                    # j-shifts (overlapping [rows, 3, 256] SBUF windows; DRAM
                    # reordered to (h, k, w)).  group 0 holds D rows i..49 ->
                    # slab rows 0..49-i; group 1 holds D rows 50..i+95 ->
                    # slab rows 50-i..95.
                    for i in range(3):
                        if g == 0:
                            rows = min(GSPLIT - i, HALF)
                            src = dpad[i : i + rows, 0:W]
                            hs = slice(0, rows)
                        else:
                            n0 = min(GSPLIT - i, HALF)
                            rows = HALF - n0
                            if rows <= 0:
                                continue
                            src = dpad[0:rows, 0:W]
                            hs = slice(n0, HALF)
                        sap = src.ap
                        src3 = bass.AP(
                            src.tensor,
                            src.offset,
                            [list(sap[0]), [1, 3], list(sap[1])],
                        )
                        dst3 = out4[i].transpose([1, 0, 2])[hs]
                        eng = nc.sync if i != 1 else nc.scalar
                        eng.dma_start(out=dst3, in_=src3)
    nc.finalize()
    return nc


def _make_shards(image: np.ndarray, event: np.ndarray):
    in_maps = []
    for c in range(N_CORES):
        b, half = divmod(c, 2)
        h0 = half * HALF
        m = {}
        for name, src in (("image", image), ("event", event)):
            shard = np.zeros((C, ROWS, W), dtype=np.float16)
            r0 = max(h0 - 1, 0)
            r1 = min(h0 + HALF + 1, H)
            d0 = r0 - (h0 - 1)
            shard[:, d0 : d0 + (r1 - r0), :] = src[b, :, r0:r1, :]
            m[name] = shard.reshape(C, ROWS * W)
        in_maps.append(m)
    return in_maps


_PROGRAM = None


def _get_program():
    global _PROGRAM
    if _PROGRAM is None:
        _PROGRAM = build_program()
    return _PROGRAM


def run(image: np.ndarray, event: np.ndarray, trace: bool = False):
    """Run on 8 cores; returns (full_output, BassKernelResults)."""
    image = np.ascontiguousarray(np.asarray(image), dtype=np.float32)
    event = np.ascontiguousarray(np.asarray(event), dtype=np.float32)
    assert image.shape == (B, C, H, W) and event.shape == (B, C, H, W)
    nc = _get_program()
    in_maps = _make_shards(image, event)
    res = run_bass_kernel_spmd(nc, in_maps, list(range(N_CORES)), trace=trace)
    full = np.empty((B, KWIN, H, W), dtype=np.float32)
    for c in range(N_CORES):
        b, half = divmod(c, 2)
        h0 = half * HALF
        full[b, :, h0 : h0 + HALF, :] = res.results[c]["out"]
    return full, res


def kernel(image: np.ndarray, event: np.ndarray) -> np.ndarray:
    out, _ = run(image, event, trace=False)
    return out
